# revision 12
# baseline (speedup 1.0000x reference)
"""ColBERT MaxSim contrastive loss on 8 Trainium2 NeuronCores.

scores[b, c] = (1/q_len[b]) * sum_n max_s <q[b, n, :], d[c, s, :]>
loss = CE(scores / T, labels=arange(B)), mean reduction.

Sharding: data-parallel over the *doc* batch dim (columns of the score
matrix). Each core holds the full query set (1 MB) plus its 8-doc shard
(4 MB) instead of the all-gathered 32 MB doc tensor, computes its
(B_global, B_local) = (64, 8) score block fully on device (fp16 matmuls
at full PE rate + fused max-reduction), and the host performs the final
gather + tiny 64x64 CE reduction (same "host sums the partials" tail as
the standard contrastive sharding).

Device pipeline per core:
  1. DMA q (64,32,128) f32 and d_shard (8,1024,128) f32 into SBUF in
     token-major layout, cast to fp16.
  2. xbar DMA-transpose 128x128 blocks into [D, token] layout (the PE
     contracts over the partition dim).
  3. For each (query-group g of 4 queries, doc c): two fp16 matmuls
     qT[128, g*128:+128].T @ dT[:, c*1024:+1024] -> PSUM [128, 1024]
     (PE at 1 cycle/row).
  4. The 1024-wide max-reduce of each PSUM set is spread over three
     engine channels (PSUM has a single DVE read port, so no one engine
     can keep up with the PE): path A = ScalarE copies bank B to SBUF,
     VectorE tensor_tensor(max) folds bank A against it; path B =
     ScalarE copies both banks to fp16 SBUF; path C = DMA evicts both
     banks to SBUF. VectorE then runs batched reduce_max over the SBUF
     staging tiles (4x mode on fp16, 2x on fp32).
  5. A tiny selector matmul sums the 32 token-maxes per query:
     out[4, 128] = sel.T @ maxes.
Host: out blocks -> scores (64, 64) -> q_len scaling -> CE loss.
"""

import json

import numpy as np

import concourse.bass as bass
import concourse.mybir as mybir
import concourse.tile as tile
from concourse.bass_utils import run_bass_kernel_spmd

B = 64          # queries (= docs, contrastive batch)
NQ = 32         # tokens per query
ND = 1024       # tokens per doc
D = 128         # embedding dim
NCORES = 8
CL = B // NCORES  # docs per core
TEMPERATURE = 0.02
NORMALIZE_SCORES = True

F32 = mybir.dt.float32
F16 = mybir.dt.float16


def _split_waits_json(bir_bytes: bytes) -> bytes:
    """Walrus in this toolchain rejects >1 sem-wait per instruction on the
    Tile end-of-kernel drain; split extra waits onto preceding Drains."""
    bir = json.loads(bir_bytes)
    for f in bir["functions"]:
        for blk in f["blocks"]:
            fixed = []
            for ins in blk["instructions"]:
                si = ins.get("sync_info") or {}
                waits = si.get("on_wait") or []
                if len(waits) > 1:
                    for i, w in enumerate(waits[:-1]):
                        fixed.append({
                            "debug": ins.get("debug", 0),
                            "engine": ins["engine"],
                            "ins": [],
                            "is_reset_sema": False,
                            "name": f'{ins["name"]}-wsplit{i}',
                            "opcode": "Drain",
                            "outs": [],
                            "sync_info": {"on_update": [], "on_wait": [w]},
                        })
                    si["on_wait"] = waits[-1:]
                    ins["sync_info"] = si
                fixed.append(ins)
            blk["instructions"] = fixed
    return json.dumps(bir).encode()


def _patch_nc(nc):
    orig = nc.to_json_bytes

    def patched(*a, **k):
        return _split_waits_json(orig(*a, **k))

    nc.to_json_bytes = patched
    return nc


# Per-doc consumption path for each query group's 8 sets. PSUM has one
# DVE read port and ACT is 1x, so the 1024-value max-reduce per set is
# split across channels:
# S: ACT copies bank B -> SBUF f32; DVE tensor_tensor_scan(max, max)
#    folds PSUM bank A + that copy in one pass (last scan column = max);
#    a tiny strided gather collects the last columns.
# B: ACT copies both banks -> fp16 SBUF; DVE batched 4x reduce_max.
# M: DVE reduce_max directly on the [128, 1024] PSUM set.
PATH_PATTERN = "SSSSSBBM"


def build_nc(path_pattern=None):
    """Build the per-core Bass program (SPMD: every core runs this; only
    the data in its "d" shard differs)."""
    pattern = path_pattern or PATH_PATTERN
    assert len(pattern) == CL and set(pattern) <= set("SBM")
    nc = bass.Bass("TRN2", target_bir_lowering=False, debug=False,
                   num_devices=NCORES)
    q_dram = nc.dram_tensor("q", [B, NQ, D], F32, kind="ExternalInput").ap()
    d_dram = nc.dram_tensor("d", [CL, ND, D], F32, kind="ExternalInput").ap()
    out_dram = nc.dram_tensor("out", [4, 128], F32, kind="ExternalOutput").ap()

    NQTOK = B * NQ          # 2048 query tokens
    NDTOK = CL * ND         # 8192 doc tokens
    NSETS = (NQTOK // 128) * CL   # 16 groups * 8 docs = 128 matmul sets

    nS = pattern.count("S")
    nB = pattern.count("B")

    with tile.TileContext(nc) as tc:
        with (
            tc.tile_pool(name="prep", bufs=1) as prep,
            tc.tile_pool(name="sbb", bufs=4) as sbb_pool,
            tc.tile_pool(name="stgs", bufs=2) as stgs_pool,
            tc.tile_pool(name="stgb", bufs=2) as stgb_pool,
            tc.tile_pool(name="mm", bufs=3, space="PSUM") as psum_pool,
            tc.tile_pool(name="selps", bufs=1, space="PSUM") as sel_psum_pool,
        ):
            # ---- prologue: load + cast + transpose ----
            q_nat = prep.tile([128, NQTOK], F32)
            # partition p = (b%4)*32 + n, free = (b//4)*128 + d
            nc.sync.dma_start(
                q_nat[:].rearrange("p (t d) -> p t d", t=NQTOK // 128),
                q_dram.rearrange("(t bb) n d -> (bb n) t d", t=NQTOK // 128),
            )
            q16 = prep.tile([128, NQTOK], F16)
            nc.vector.tensor_copy(q16[:], q_nat[:])
            qT = prep.tile([128, NQTOK], F16)
            for j in range(NQTOK // 128):
                nc.sync.dma_start_transpose(
                    qT[:, bass.ts(j, 128)], q16[:, bass.ts(j, 128)])

            d_nat = prep.tile([128, NDTOK], F32)
            nc.sync.dma_start(
                d_nat[:].rearrange("p (t d) -> p t d", t=NDTOK // 128),
                d_dram.rearrange("c (t p) d -> p (c t) d", p=128),
            )
            d16 = prep.tile([128, NDTOK], F16)
            nc.vector.tensor_copy(d16[:], d_nat[:])
            dT = prep.tile([128, NDTOK], F16)
            for j in range(NDTOK // 128):
                nc.sync.dma_start_transpose(
                    dT[:, bass.ts(j, 128)], d16[:, bass.ts(j, 128)])

            # selector: sel[p, m] = 1 if p//32 == m (sums tokens per query)
            sel = prep.tile([128, 4], F32)
            nc.gpsimd.memset(sel[:], 0.0)
            for m in range(4):
                nc.gpsimd.memset(sel[32 * m:32 * (m + 1), m:m + 1], 1.0)

            maxes = prep.tile([128, NSETS], F32)

            # ---- main loop: 16 query groups x 8 docs ----
            for g in range(NQTOK // 128):
                # staging tiles for this query group's batched reductions
                stgs = stgs_pool.tile([128, nS * 512], F16, tag="stgs",
                                      name="stgs") if nS else None
                stgb = stgb_pool.tile([128, nB * 1024], F16, tag="stgb",
                                      name="stgb") if nB else None
                i_s = i_b = 0
                s_idx, b_idx = [], []
                lhs = qT[:, bass.ts(g, 128)]
                for c in range(CL):
                    idx = g * CL + c
                    path = pattern[c]
                    pa = psum_pool.tile([128, 1024], F32)
                    nc.tensor.matmul(pa[:, 0:512], lhs,
                                     dT[:, c * ND:c * ND + 512],
                                     start=True, stop=True)
                    nc.tensor.matmul(pa[:, 512:1024], lhs,
                                     dT[:, c * ND + 512:c * ND + 1024],
                                     start=True, stop=True)
                    if path == "S":
                        sbb = sbb_pool.tile([128, 512], F32)
                        nc.scalar.copy(sbb[:], pa[:, 512:1024])
                        nc.vector.tensor_tensor_scan(
                            out=stgs[:, bass.ts(i_s, 512)],
                            data0=pa[:, 0:512], data1=sbb[:],
                            initial=-1e30,
                            op0=mybir.AluOpType.max, op1=mybir.AluOpType.max)
                        s_idx.append(idx)
                        i_s += 1
                    elif path == "B":
                        nc.scalar.copy(stgb[:, bass.ts(i_b, 1024)], pa[:])
                        b_idx.append(idx)
                        i_b += 1
                    else:  # M: DVE consumes the whole set from PSUM
                        nc.vector.reduce_max(maxes[:, idx:idx + 1], pa[:],
                                             axis=mybir.AxisListType.X)
                # gather scan tails / batched reductions into maxes columns
                if nS:
                    assert s_idx == list(range(s_idx[0], s_idx[0] + nS))
                    tails = stgs[:].rearrange("p (s f) -> p s f", s=nS)
                    nc.vector.tensor_copy(
                        maxes[:, s_idx[0]:s_idx[0] + nS],
                        tails[:, :, 511:512].rearrange("p s one -> p (s one)"))
                if nB:
                    assert b_idx == list(range(b_idx[0], b_idx[0] + nB))
                    nc.vector.reduce_max(
                        maxes[:, b_idx[0]:b_idx[0] + nB],
                        stgb[:].rearrange("p (s f) -> p s f", s=nB),
                        axis=mybir.AxisListType.X)

            # ---- reduce over the 32 tokens of each query ----
            sel_ps = sel_psum_pool.tile([4, NSETS], F32)
            nc.tensor.matmul(sel_ps[:], sel[:], maxes[:], start=True, stop=True)
            out_sb = prep.tile([4, NSETS], F32)
            nc.vector.tensor_copy(out_sb[:], sel_ps[:])
            nc.sync.dma_start(out_dram, out_sb[:])

    nc.finalize()
    return _patch_nc(nc)


_NC = None


def _get_nc():
    global _NC
    if _NC is None:
        _NC = build_nc()
    return _NC


def assemble_loss(outs, q):
    """Host tail: per-core [4, 128] blocks -> scores -> CE loss."""
    scores = np.zeros((B, B), np.float64)
    for k in range(NCORES):
        blk = np.asarray(outs[k], np.float64)  # [m=4, idx=g*8+c]
        for g in range(B // 4):
            for m in range(4):
                for c in range(CL):
                    scores[4 * g + m, CL * k + c] = blk[m, g * CL + c]
    if NORMALIZE_SCORES:
        q_len = (np.asarray(q)[:, :, 0] != 0).sum(axis=1).astype(np.float64)
        scores = scores / q_len[:, None]
    logits = scores / TEMPERATURE
    m = logits.max(axis=1, keepdims=True)
    logz = m[:, 0] + np.log(np.exp(logits - m).sum(axis=1))
    loss = -(np.diag(logits) - logz).mean()
    return np.float32(loss)


def kernel(query_embeddings, doc_embeddings):
    q = np.ascontiguousarray(np.asarray(query_embeddings, dtype=np.float32))
    d = np.ascontiguousarray(np.asarray(doc_embeddings, dtype=np.float32))
    nc = _get_nc()
    in_maps = [
        {"q": q, "d": np.ascontiguousarray(d[CL * k:CL * (k + 1)])}
        for k in range(NCORES)
    ]
    res = run_bass_kernel_spmd(nc, in_maps, core_ids=list(range(NCORES)))
    outs = [res.results[k]["out"] for k in range(NCORES)]
    return assemble_loss(outs, q)


# revision 13
# speedup vs baseline: 1.2915x; 1.2915x over previous
"""ColBERT MaxSim contrastive loss on 8 Trainium2 NeuronCores.

scores[b, c] = (1/q_len[b]) * sum_n max_s <q[b, n, :], d[c, s, :]>
loss = CE(scores / T, labels=arange(B)), mean reduction.

Sharding: data-parallel over the *doc* batch dim (columns of the score
matrix). Each core holds the full query set (1 MB) plus its 8-doc shard
(4 MB) instead of the all-gathered 32 MB doc tensor, computes its
(B_global, B_local) = (64, 8) score block fully on device (fp16 matmuls
at full PE rate + fused max-reduction), and the host performs the final
gather + tiny 64x64 CE reduction (same "host sums the partials" tail as
the standard contrastive sharding).

Device pipeline per core:
  1. DMA q (64,32,128) f32 and d_shard (8,1024,128) f32 into SBUF in
     token-major layout, cast to fp16.
  2. xbar DMA-transpose 128x128 blocks into [D, token] layout (the PE
     contracts over the partition dim).
  3. For each (query-group g of 4 queries, doc c): two fp16 matmuls
     qT[128, g*128:+128].T @ dT[:, c*1024:+1024] -> PSUM [128, 1024]
     (PE at 1 cycle/row).
  4. The 1024-wide max-reduce of each PSUM set is spread over three
     engine channels (PSUM has a single DVE read port, so no one engine
     can keep up with the PE): path A = ScalarE copies bank B to SBUF,
     VectorE tensor_tensor(max) folds bank A against it; path B =
     ScalarE copies both banks to fp16 SBUF; path C = DMA evicts both
     banks to SBUF. VectorE then runs batched reduce_max over the SBUF
     staging tiles (4x mode on fp16, 2x on fp32).
  5. A tiny selector matmul sums the 32 token-maxes per query:
     out[4, 128] = sel.T @ maxes.
Host: out blocks -> scores (64, 64) -> q_len scaling -> CE loss.
"""

import json

import numpy as np

import concourse.bass as bass
import concourse.mybir as mybir
import concourse.tile as tile
from concourse.bass_utils import run_bass_kernel_spmd

B = 64          # queries (= docs, contrastive batch)
NQ = 32         # tokens per query
ND = 1024       # tokens per doc
D = 128         # embedding dim
NCORES = 8
CL = B // NCORES  # docs per core
TEMPERATURE = 0.02
NORMALIZE_SCORES = True

F32 = mybir.dt.float32
F16 = mybir.dt.float16


def _split_waits_json(bir_bytes: bytes) -> bytes:
    """Walrus in this toolchain rejects >1 sem-wait per instruction on the
    Tile end-of-kernel drain; split extra waits onto preceding Drains."""
    bir = json.loads(bir_bytes)
    for f in bir["functions"]:
        for blk in f["blocks"]:
            fixed = []
            for ins in blk["instructions"]:
                si = ins.get("sync_info") or {}
                waits = si.get("on_wait") or []
                if len(waits) > 1:
                    for i, w in enumerate(waits[:-1]):
                        fixed.append({
                            "debug": ins.get("debug", 0),
                            "engine": ins["engine"],
                            "ins": [],
                            "is_reset_sema": False,
                            "name": f'{ins["name"]}-wsplit{i}',
                            "opcode": "Drain",
                            "outs": [],
                            "sync_info": {"on_update": [], "on_wait": [w]},
                        })
                    si["on_wait"] = waits[-1:]
                    ins["sync_info"] = si
                fixed.append(ins)
            blk["instructions"] = fixed
    return json.dumps(bir).encode()


def _patch_nc(nc):
    orig = nc.to_json_bytes

    def patched(*a, **k):
        return _split_waits_json(orig(*a, **k))

    nc.to_json_bytes = patched
    return nc


# Per-doc consumption path for each query group's 8 sets. PSUM has one
# DVE read port and ACT is 1x, so the 1024-value max-reduce per set is
# split across channels:
# S: ACT copies bank B -> SBUF f32; DVE tensor_tensor_scan(max, max)
#    folds PSUM bank A + that copy in one pass (last scan column = max);
#    a tiny strided gather collects the last columns.
# B: ACT copies both banks -> fp16 SBUF; DVE batched 4x reduce_max.
# M: DVE reduce_max directly on the [128, 1024] PSUM set.
PATH_PATTERN = "AAAAAAAB"


def build_nc(path_pattern=None):
    """Build the per-core Bass program (SPMD: every core runs this; only
    the data in its "d" shard differs)."""
    pattern = path_pattern or PATH_PATTERN
    assert len(pattern) == CL and set(pattern) <= set("ABM")
    nc = bass.Bass("TRN2", target_bir_lowering=False, debug=False,
                   num_devices=NCORES)
    q_dram = nc.dram_tensor("q", [B, NQ, D], F32, kind="ExternalInput").ap()
    d_dram = nc.dram_tensor("d", [CL, ND, D], F32, kind="ExternalInput").ap()
    out_dram = nc.dram_tensor("out", [4, 128], F32, kind="ExternalOutput").ap()

    NQTOK = B * NQ          # 2048 query tokens
    NDTOK = CL * ND         # 8192 doc tokens
    NSETS = (NQTOK // 128) * CL   # 16 groups * 8 docs = 128 matmul sets

    nS = pattern.count("A")
    nB = pattern.count("B")

    with tile.TileContext(nc) as tc:
        with (
            tc.tile_pool(name="prep", bufs=1) as prep,
            tc.tile_pool(name="sbb", bufs=4) as sbb_pool,
            tc.tile_pool(name="stgs", bufs=2) as stgs_pool,
            tc.tile_pool(name="stgb", bufs=2) as stgb_pool,
            tc.tile_pool(name="mm", bufs=3, space="PSUM") as psum_pool,
            tc.tile_pool(name="selps", bufs=1, space="PSUM") as sel_psum_pool,
        ):
            # ---- prologue: load + cast + transpose ----
            q_nat = prep.tile([128, NQTOK], F32)
            # partition p = (b%4)*32 + n, free = (b//4)*128 + d
            nc.sync.dma_start(
                q_nat[:].rearrange("p (t d) -> p t d", t=NQTOK // 128),
                q_dram.rearrange("(t bb) n d -> (bb n) t d", t=NQTOK // 128),
            )
            q16 = prep.tile([128, NQTOK], F16)
            nc.vector.tensor_copy(q16[:], q_nat[:])
            qT = prep.tile([128, NQTOK], F16)
            nc.sync.dma_start_transpose(
                qT[:].rearrange("p (t f) -> p t f", t=NQTOK // 128), q16[:])

            d_nat = prep.tile([128, NDTOK], F32)
            nc.sync.dma_start(
                d_nat[:].rearrange("p (t d) -> p t d", t=NDTOK // 128),
                d_dram.rearrange("c (t p) d -> p (c t) d", p=128),
            )
            d16 = prep.tile([128, NDTOK], F16)
            nc.vector.tensor_copy(d16[:], d_nat[:])
            dT = prep.tile([128, NDTOK], F16)
            nc.sync.dma_start_transpose(
                dT[:].rearrange("p (t f) -> p t f", t=NDTOK // 128), d16[:])

            # selector: sel[p, m] = 1 if p//32 == m (sums tokens per query)
            sel = prep.tile([128, 4], F32)
            nc.gpsimd.memset(sel[:], 0.0)
            for m in range(4):
                nc.gpsimd.memset(sel[32 * m:32 * (m + 1), m:m + 1], 1.0)

            maxes = prep.tile([128, NSETS], F32)

            # ---- main loop: 16 query groups x 8 docs ----
            for g in range(NQTOK // 128):
                # staging tiles for this query group's batched reductions
                stgs = stgs_pool.tile([128, nS * 512], F16, tag="stgs",
                                      name="stgs") if nS else None
                stgb = stgb_pool.tile([128, nB * 1024], F16, tag="stgb",
                                      name="stgb") if nB else None
                i_s = i_b = 0
                s_idx, b_idx = [], []
                lhs = qT[:, bass.ts(g, 128)]
                for c in range(CL):
                    idx = g * CL + c
                    path = pattern[c]
                    pa = psum_pool.tile([128, 1024], F32)
                    nc.tensor.matmul(pa[:, 0:512], lhs,
                                     dT[:, c * ND:c * ND + 512],
                                     start=True, stop=True)
                    nc.tensor.matmul(pa[:, 512:1024], lhs,
                                     dT[:, c * ND + 512:c * ND + 1024],
                                     start=True, stop=True)
                    if path == "A":
                        sbb = sbb_pool.tile([128, 512], F32)
                        nc.scalar.copy(sbb[:], pa[:, 512:1024])
                        nc.vector.tensor_tensor(
                            out=stgs[:, bass.ts(i_s, 512)],
                            in0=pa[:, 0:512], in1=sbb[:],
                            op=mybir.AluOpType.max)
                        s_idx.append(idx)
                        i_s += 1
                    elif path == "B":
                        nc.scalar.copy(stgb[:, bass.ts(i_b, 1024)], pa[:])
                        b_idx.append(idx)
                        i_b += 1
                    else:  # M: DVE consumes the whole set from PSUM
                        nc.vector.reduce_max(maxes[:, idx:idx + 1], pa[:],
                                             axis=mybir.AxisListType.X)
                # batched second-level reductions into maxes columns
                if nS:
                    assert s_idx == list(range(s_idx[0], s_idx[0] + nS))
                    nc.vector.reduce_max(
                        maxes[:, s_idx[0]:s_idx[0] + nS],
                        stgs[:].rearrange("p (s f) -> p s f", s=nS),
                        axis=mybir.AxisListType.X)
                if nB:
                    assert b_idx == list(range(b_idx[0], b_idx[0] + nB))
                    nc.vector.reduce_max(
                        maxes[:, b_idx[0]:b_idx[0] + nB],
                        stgb[:].rearrange("p (s f) -> p s f", s=nB),
                        axis=mybir.AxisListType.X)

            # ---- reduce over the 32 tokens of each query ----
            sel_ps = sel_psum_pool.tile([4, NSETS], F32)
            nc.tensor.matmul(sel_ps[:], sel[:], maxes[:], start=True, stop=True)
            out_sb = prep.tile([4, NSETS], F32)
            nc.vector.tensor_copy(out_sb[:], sel_ps[:])
            nc.sync.dma_start(out_dram, out_sb[:])

    nc.finalize()
    return _patch_nc(nc)


_NC = None


def _get_nc():
    global _NC
    if _NC is None:
        _NC = build_nc()
    return _NC


def assemble_loss(outs, q):
    """Host tail: per-core [4, 128] blocks -> scores -> CE loss."""
    scores = np.zeros((B, B), np.float64)
    for k in range(NCORES):
        blk = np.asarray(outs[k], np.float64)  # [m=4, idx=g*8+c]
        for g in range(B // 4):
            for m in range(4):
                for c in range(CL):
                    scores[4 * g + m, CL * k + c] = blk[m, g * CL + c]
    if NORMALIZE_SCORES:
        q_len = (np.asarray(q)[:, :, 0] != 0).sum(axis=1).astype(np.float64)
        scores = scores / q_len[:, None]
    logits = scores / TEMPERATURE
    m = logits.max(axis=1, keepdims=True)
    logz = m[:, 0] + np.log(np.exp(logits - m).sum(axis=1))
    loss = -(np.diag(logits) - logz).mean()
    return np.float32(loss)


def kernel(query_embeddings, doc_embeddings):
    q = np.ascontiguousarray(np.asarray(query_embeddings, dtype=np.float32))
    d = np.ascontiguousarray(np.asarray(doc_embeddings, dtype=np.float32))
    nc = _get_nc()
    in_maps = [
        {"q": q, "d": np.ascontiguousarray(d[CL * k:CL * (k + 1)])}
        for k in range(NCORES)
    ]
    res = run_bass_kernel_spmd(nc, in_maps, core_ids=list(range(NCORES)))
    outs = [res.results[k]["out"] for k in range(NCORES)]
    return assemble_loss(outs, q)


# revision 14
# speedup vs baseline: 1.3603x; 1.0533x over previous
"""ColBERT MaxSim contrastive loss on 8 Trainium2 NeuronCores.

scores[b, c] = (1/q_len[b]) * sum_n max_s <q[b, n, :], d[c, s, :]>
loss = CE(scores / T, labels=arange(B)), mean reduction.

Sharding: data-parallel over the *doc* batch dim (columns of the score
matrix). Each core holds the full query set (1 MB) plus its 8-doc shard
(4 MB) instead of the all-gathered 32 MB doc tensor, computes its
(B_global, B_local) = (64, 8) score block fully on device (fp16 matmuls
at full PE rate + split max-reduction), and the host performs the final
gather + tiny 64x64 CE reduction (the same "host sums the partials"
tail as the standard contrastive sharding).

Device pipeline per core:
  1. DMA q (64,32,128) f32 and d_shard (8,1024,128) f32 into SBUF in
     token-major layout (per doc-pair, so the main loop starts while
     later pairs still load), cast to fp16.
  2. One batched xbar DMA-transpose per tensor/pair flips 128x128
     blocks into [D, token] layout (the PE contracts over partitions).
  3. For each (query group g of 4 queries, doc pair): four fp16
     matmuls -> one [128, 2048] PSUM tile (two docs x 1024 tokens).
  4. Max-reduce consumption, split by measured engine rates (PSUM has
     one DVE read port; DVE reduce-from-PSUM ~1.5 cyc/elem, ACT copy
     ~1 cyc/elem):
       path M: DVE reduce_max directly on PSUM -> maxes columns.
       path B: ACT copies the PSUM tile to fp16 SBUF; DVE runs a 4x
               batched reduce_max over the staged fp16.
     ~28% of sets go to M, the rest to B, balancing DVE and ACT.
  5. A tiny selector matmul sums the 32 token-maxes per query:
     out[4, 128] = sel.T @ maxes.
Host: out blocks -> scores (64, 64) -> q_len scaling -> CE loss.
"""

import json

import numpy as np

import concourse.bass as bass
import concourse.mybir as mybir
import concourse.tile as tile
from concourse.bass_utils import run_bass_kernel_spmd

B = 64          # queries (= docs, contrastive batch)
NQ = 32         # tokens per query
ND = 1024       # tokens per doc
D = 128         # embedding dim
NCORES = 8
CL = B // NCORES  # docs per core
TEMPERATURE = 0.02
NORMALIZE_SCORES = True

F32 = mybir.dt.float32
F16 = mybir.dt.float16

NG = (B * NQ) // 128        # 16 query groups of 4 queries
NPAIR = CL // 2             # 4 doc pairs per core
NSETS = NG * CL             # 128 (query group, doc) sets

# Every g: doc pair 0 -> path M; every M_EXTRA_PERIOD-th g additionally
# sends pair 1 to M. Tuned so DVE ~= ACT busy time.
M_EXTRA_PERIOD = 8


def _split_waits_json(bir_bytes: bytes) -> bytes:
    """Walrus in this toolchain rejects >1 sem-wait per instruction on the
    Tile end-of-kernel drain; split extra waits onto preceding Drains."""
    bir = json.loads(bir_bytes)
    for f in bir["functions"]:
        for blk in f["blocks"]:
            fixed = []
            for ins in blk["instructions"]:
                si = ins.get("sync_info") or {}
                waits = si.get("on_wait") or []
                if len(waits) > 1:
                    for i, w in enumerate(waits[:-1]):
                        fixed.append({
                            "debug": ins.get("debug", 0),
                            "engine": ins["engine"],
                            "ins": [],
                            "is_reset_sema": False,
                            "name": f'{ins["name"]}-wsplit{i}',
                            "opcode": "Drain",
                            "outs": [],
                            "sync_info": {"on_update": [], "on_wait": [w]},
                        })
                    si["on_wait"] = waits[-1:]
                    ins["sync_info"] = si
                fixed.append(ins)
            blk["instructions"] = fixed
    return json.dumps(bir).encode()


def _patch_nc(nc):
    orig = nc.to_json_bytes

    def patched(*a, **k):
        return _split_waits_json(orig(*a, **k))

    nc.to_json_bytes = patched
    return nc


def build_nc(m_extra_period=None):
    """Build the per-core Bass program (SPMD: every core runs this; only
    the data in its "d" shard differs)."""
    mper = m_extra_period or M_EXTRA_PERIOD
    nc = bass.Bass("TRN2", target_bir_lowering=False, debug=False,
                   num_devices=NCORES)
    q_dram = nc.dram_tensor("q", [B, NQ, D], F32, kind="ExternalInput").ap()
    d_dram = nc.dram_tensor("d", [CL, ND, D], F32, kind="ExternalInput").ap()
    out_dram = nc.dram_tensor("out", [4, NSETS], F32, kind="ExternalOutput").ap()

    NQTOK = B * NQ          # 2048 query tokens

    with tile.TileContext(nc) as tc:
        with (
            tc.tile_pool(name="prep", bufs=1) as prep,
            tc.tile_pool(name="dload", bufs=2) as dload_pool,
            tc.tile_pool(name="stgb", bufs=2) as stgb_pool,
            tc.tile_pool(name="mm", bufs=2, space="PSUM") as psum_pool,
        ):
            # ---- prologue: load + cast + transpose ----
            q_nat = prep.tile([128, NQTOK], F32)
            # partition p = (b%4)*32 + n, free = (b//4)*128 + d
            nc.sync.dma_start(
                q_nat[:].rearrange("p (t d) -> p t d", t=NQTOK // 128),
                q_dram.rearrange("(t bb) n d -> (bb n) t d", t=NQTOK // 128),
            )
            q16 = prep.tile([128, NQTOK], F16)
            nc.vector.tensor_copy(q16[:], q_nat[:])
            qT = prep.tile([128, NQTOK], F16)
            nc.sync.dma_start_transpose(
                qT[:].rearrange("p (t f) -> p t f", t=NQTOK // 128), q16[:])

            # d, one doc pair at a time so compute overlaps later loads
            dT = []
            for p in range(NPAIR):
                d_nat = dload_pool.tile([128, 2048], F32, tag="dnat",
                                        name="dnat")
                nc.sync.dma_start(
                    d_nat[:].rearrange("p (t d) -> p t d", t=16),
                    d_dram[2 * p:2 * p + 2].rearrange(
                        "c (t p) d -> p (c t) d", p=128),
                )
                d16 = dload_pool.tile([128, 2048], F16, tag="d16", name="d16")
                nc.vector.tensor_copy(d16[:], d_nat[:])
                dTp = prep.tile([128, 2048], F16, tag=f"dT{p}", name=f"dT{p}")
                nc.sync.dma_start_transpose(
                    dTp[:].rearrange("p (t f) -> p t f", t=16), d16[:])
                dT.append(dTp)

            # selector: sel[p, m] = 1 if p//32 == m (sums tokens per query)
            sel = prep.tile([128, 4], F32)
            nc.gpsimd.memset(sel[:], 0.0)
            for m in range(4):
                nc.gpsimd.memset(sel[32 * m:32 * (m + 1), m:m + 1], 1.0)

            maxes = prep.tile([128, NSETS], F32)

            # ---- main loop: 16 query groups x 4 doc pairs ----
            for g in range(NG):
                n_m_pairs = 2 if (g % mper == 0) else 1
                n_b = NPAIR - n_m_pairs
                stgb = stgb_pool.tile([128, n_b * 2048], F16, tag="stgb",
                                      name="stgb") if n_b else None
                i_b = 0
                b_idx = []
                lhs = qT[:, bass.ts(g, 128)]
                for p in range(NPAIR):
                    idx = g * CL + 2 * p      # maxes column of doc 2p
                    pa = psum_pool.tile([128, 2048], F32, tag="pa", name="pa")
                    for cc in range(2):
                        base = cc * 1024
                        nc.tensor.matmul(
                            pa[:, base:base + 512], lhs,
                            dT[p][:, cc * 1024:cc * 1024 + 512],
                            start=True, stop=True)
                        nc.tensor.matmul(
                            pa[:, base + 512:base + 1024], lhs,
                            dT[p][:, cc * 1024 + 512:cc * 1024 + 1024],
                            start=True, stop=True)
                    if p < n_m_pairs:
                        # path M: DVE reduces both docs straight from PSUM
                        nc.vector.reduce_max(
                            maxes[:, idx:idx + 2],
                            pa[:].rearrange("p (c f) -> p c f", c=2),
                            axis=mybir.AxisListType.X)
                    else:
                        # path B: ACT stages to fp16; DVE reduces later
                        nc.scalar.copy(stgb[:, bass.ts(i_b, 2048)], pa[:])
                        b_idx.extend([idx, idx + 1])
                        i_b += 1
                if n_b:
                    assert b_idx == list(range(b_idx[0], b_idx[0] + 2 * n_b))
                    nc.vector.reduce_max(
                        maxes[:, b_idx[0]:b_idx[0] + 2 * n_b],
                        stgb[:].rearrange("p (s f) -> p s f", s=2 * n_b),
                        axis=mybir.AxisListType.X)

            # ---- reduce over the 32 tokens of each query ----
            sel_ps = psum_pool.tile([4, NSETS], F32, tag="pa", name="selps")
            nc.tensor.matmul(sel_ps[:], sel[:], maxes[:], start=True, stop=True)
            out_sb = prep.tile([4, NSETS], F32)
            nc.vector.tensor_copy(out_sb[:], sel_ps[:])
            nc.sync.dma_start(out_dram, out_sb[:])

    nc.finalize()
    return _patch_nc(nc)


_NC = None


def _get_nc():
    global _NC
    if _NC is None:
        _NC = build_nc()
    return _NC


def assemble_loss(outs, q):
    """Host tail: per-core [4, 128] blocks -> scores -> CE loss."""
    scores = np.zeros((B, B), np.float64)
    for k in range(NCORES):
        blk = np.asarray(outs[k], np.float64)  # [m=4, idx=g*8+c]
        for g in range(B // 4):
            for m in range(4):
                for c in range(CL):
                    scores[4 * g + m, CL * k + c] = blk[m, g * CL + c]
    if NORMALIZE_SCORES:
        q_len = (np.asarray(q)[:, :, 0] != 0).sum(axis=1).astype(np.float64)
        scores = scores / q_len[:, None]
    logits = scores / TEMPERATURE
    m = logits.max(axis=1, keepdims=True)
    logz = m[:, 0] + np.log(np.exp(logits - m).sum(axis=1))
    loss = -(np.diag(logits) - logz).mean()
    return np.float32(loss)


def kernel(query_embeddings, doc_embeddings):
    q = np.ascontiguousarray(np.asarray(query_embeddings, dtype=np.float32))
    d = np.ascontiguousarray(np.asarray(doc_embeddings, dtype=np.float32))
    nc = _get_nc()
    in_maps = [
        {"q": q, "d": np.ascontiguousarray(d[CL * k:CL * (k + 1)])}
        for k in range(NCORES)
    ]
    res = run_bass_kernel_spmd(nc, in_maps, core_ids=list(range(NCORES)))
    outs = [res.results[k]["out"] for k in range(NCORES)]
    return assemble_loss(outs, q)
